# revision 28
# baseline (speedup 1.0000x reference)
"""Self-contained ChildSum TreeLSTM kernel for 8 Trainium2 NeuronCores.

Strategy v2: subtree-partitioned data parallelism with fully SBUF-resident
node states and matmul-based child aggregation.

  - Bulk nodes (small subtrees) are packed into 8 per-core forests; the top
    "tail" nodes (level >= cut or huge subtree) are replicated on every core.
    Each core computes partial child-sums for tail nodes from its own bulk
    children (pseudo phase), one AllReduce combines them, then every core
    runs the tail levels identically.
  - Node states [fh | c | h] (fh = h @ W_fh.T) live in SBUF for the whole
    kernel (bf16).  Child gathers/scatters are 0/1 incidence matmuls; the
    incidence matrices are built on-device with one tensor_scalar(is_equal)
    op each from tiny per-tile index tables.
  - Slots are assigned top-down sorted by parent slot, so each parent tile's
    children form few contiguous runs -> few (edge-tile, segment) pairs.
  - All GEMMs in bf16 with f32 PSUM accumulation.

kernel(**inputs) takes full unsharded inputs, returns full [N, 150] f32 h.
"""
from contextlib import ExitStack

import numpy as np

P = 128
KC = 3          # K chunks for the 301-row contraction (padded to 384)
KA = 301        # 300 features + bias row
KPAD = KC * P   # 384


class Plan:
    pass


def _tiles(n):
    return (n + P - 1) // P


# ---------------------------------------------------------------------------
# schedule / partition
# ---------------------------------------------------------------------------

def build_plan(parents, n_cores=8):
    parents = np.asarray(parents, dtype=np.int64)
    N = parents.shape[0]

    level = np.zeros(N, np.int64)
    size = np.ones(N, np.int64)
    for i in range(N - 1):
        p = parents[i]
        if level[i] + 1 > level[p]:
            level[p] = level[i] + 1
        size[p] += size[i]
    height = int(level.max())

    # children CSR (children of j sorted ascending)
    order_by_parent = np.argsort(parents[: N - 1], kind="stable")
    csr_off = np.zeros(N + 2, np.int64)
    np.add.at(csr_off, parents[: N - 1] + 1, 1)
    csr_off = np.cumsum(csr_off)
    csr_children = order_by_parent

    def children_of(j):
        return csr_children[csr_off[j]: csr_off[j + 1]]

    # --- tail cut
    counts = np.bincount(level, minlength=height + 1)
    level_cut = height + 1
    for l in range(height + 1):
        if counts[l] <= n_cores * 40:
            level_cut = l
            break
    size_cap = max(256, int(N // (n_cores * 1.35)))
    is_tail = (level >= level_cut) | (size > size_cap)
    tail_nodes = np.nonzero(is_tail)[0]
    bulk_mask = ~is_tail

    # --- subtree roots -> LPT binning
    sub_roots = [i for i in np.nonzero(bulk_mask)[0]
                 if parents[i] == N or is_tail[parents[i]]]
    root_of = np.full(N, -1, np.int64)
    for r in sub_roots:
        root_of[r] = r
    for i in range(N - 2, -1, -1):
        if bulk_mask[i] and root_of[i] == -1:
            root_of[i] = root_of[parents[i]]
    sub_roots_arr = np.array(sub_roots, np.int64)
    order = np.argsort(-size[sub_roots_arr], kind="stable")
    bin_tot = np.zeros(n_cores, np.int64)
    bin_of_root = {}
    for k in order:
        b = int(np.argmin(bin_tot))
        bin_of_root[int(sub_roots_arr[k])] = b
        bin_tot[b] += size[sub_roots_arr[k]]
    core_of = np.full(N, -1, np.int8)
    bm = np.nonzero(bulk_mask)[0]
    core_of[bm] = [bin_of_root[int(root_of[i])] for i in bm]

    # --- slot assignment, levels descending (tail levels first, then bulk)
    tail_levels = sorted({int(level[i]) for i in tail_nodes}, reverse=True)
    bulk_levels_desc = list(range(level_cut - 1, -1, -1))

    slot_of = [np.full(N, -1, np.int64) for _ in range(n_cores)]
    # big offset so bulk parents sort after tail parents
    BIG = 1 << 40

    def parent_key(c, i):
        p = parents[i]
        if p >= N:
            return -1
        if is_tail[p]:
            return slot_of[0][p]          # tail slots shared across cores
        return BIG + slot_of[c][p]

    tail_tile_rows = []   # list of (level, [node rows]) per tail tile, ascending later
    cur = 0
    tail_level_meta = []  # (level, base_slot, n_tiles, rows list)
    for l in tail_levels:
        ns = sorted(int(i) for i in tail_nodes if level[i] == l)
        ns.sort(key=lambda i: parent_key(0, i))
        for r, i in enumerate(ns):
            for c in range(n_cores):
                slot_of[c][i] = cur + r
        nt = _tiles(len(ns))
        tail_level_meta.append((l, cur, nt, ns))
        cur += nt * P
    S_tail = cur
    # compact rank (by slot order) for the partial/reduced buffers
    crank = {}
    tail_rbase = []
    cr = 0
    for (l, base, nt, ns) in tail_level_meta:
        for t in range(nt):
            tail_rbase.append(cr)
            for i in ns[t * P:(t + 1) * P]:
                crank[int(i)] = cr
                cr += 1
    C_tail = cr

    bulk_level_meta = []  # (level, base_slot, n_tiles, rows per core)
    core_level_nodes = {}
    for l in bulk_levels_desc:
        percore = []
        mx = 0
        for c in range(n_cores):
            ns = [int(i) for i in bm if core_of[i] == c and level[i] == l]
            ns.sort(key=lambda i: parent_key(c, i))
            percore.append(ns)
            mx = max(mx, len(ns))
        if mx == 0:
            continue
        for c in range(n_cores):
            for r, i in enumerate(percore[c]):
                slot_of[c][i] = cur + r
        nt = _tiles(mx)
        bulk_level_meta.append((l, cur, nt, percore))
        cur += nt * P
    S_all = cur

    # --- pseudo parents (boundary): tail nodes with bulk children in core c
    bnd = [dict() for _ in range(n_cores)]
    for i in bm:
        p = int(parents[i])
        if p < N and is_tail[p]:
            bnd[core_of[i]].setdefault(p, []).append(int(i))
    pseudo_nodes = []
    for c in range(n_cores):
        ks = sorted(bnd[c].keys(), key=lambda j: slot_of[0][j])
        pseudo_nodes.append([(j, sorted(bnd[c][j])) for j in ks])
    n_pseudo_tiles = max((_tiles(len(pn)) for pn in pseudo_nodes), default=0)

    # ------------------------------------------------------------------
    # tile stream construction (compute order: bulk asc, pseudo, tail asc)
    # ------------------------------------------------------------------
    tables = [[] for _ in range(n_cores)]   # list of [128] f32 arrays per core

    def alloc_col(vals_per_core):
        col = len(tables[0])
        for c in range(n_cores):
            tables[c].append(vals_per_core[c])
        return col

    xp_cols = []      # per x_tp tile: per-core [128] node ids (-1 absent)
    xe_cols = []      # per x_te tile: per-core [128] node ids (parent of edge)
    scat_cols = []    # per pseudo tile: per-core [128] int32 partial rows

    tiles = []

    def build_edge_struct(edges_pc, pseudo=False):
        """edges_pc: per-core list of (child_slot, parent_row, parent_node).
        Returns list of ets."""
        net = max((_tiles(len(e)) for e in edges_pc), default=0)
        ets = []
        for k in range(net):
            chunk = [e[k * P:(k + 1) * P] for e in edges_pc]
            segs = sorted({cs // P for c in range(n_cores)
                           for (cs, _, _) in chunk[c]})
            pairs = []
            for seg in segs:
                vals = []
                for c in range(n_cores):
                    v = np.full(P, -1.0, np.float32)
                    for e, (cs, _, _) in enumerate(chunk[c]):
                        if cs // P == seg:
                            v[cs % P] = e
                    vals.append(v)
                pairs.append((seg, alloc_col(vals)))
            pt = []
            xe = []
            for c in range(n_cores):
                v = np.full(P, -1.0, np.float32)
                xv = np.full(P, -1, np.int64)
                for e, (cs, prow, pnode) in enumerate(chunk[c]):
                    v[e] = prow
                    xv[e] = pnode
                pt.append(v)
                xe.append(xv)
            ptab_col = alloc_col(pt)
            xe_idx = len(xe_cols)
            xe_cols.append(xe)
            ets.append(dict(xe_idx=xe_idx, ptab_col=ptab_col, pairs=pairs))
        return ets

    def build_h_pairs(rows_pc):
        """rows_pc: per-core list of node ids (tile rows). h_pairs over all
        children (child_slot -> parent_row)."""
        allpairs = {}
        for c in range(n_cores):
            for prow, node in enumerate(rows_pc[c]):
                if node < 0:
                    continue
                for ch in children_of(node):
                    ch = int(ch)
                    if is_tail[node] and not is_tail[ch]:
                        continue  # bulk children of tail handled via pseudo
                    seg = int(slot_of[c][ch]) // P
                    allpairs.setdefault(seg, None)
        h_pairs = []
        for seg in sorted(allpairs.keys()):
            vals = []
            for c in range(n_cores):
                v = np.full(P, -1.0, np.float32)
                for prow, node in enumerate(rows_pc[c]):
                    if node < 0:
                        continue
                    for ch in children_of(node):
                        ch = int(ch)
                        if is_tail[node] and not is_tail[ch]:
                            continue
                        s = int(slot_of[c][ch])
                        if s // P == seg:
                            v[s % P] = prow
                vals.append(v)
            h_pairs.append((seg, alloc_col(vals)))
        return h_pairs

    # ---- bulk levels ascending
    for (l, base, nt, percore) in reversed(bulk_level_meta):
        for t in range(nt):
            rows_pc = []
            for c in range(n_cores):
                ns = percore[c][t * P:(t + 1) * P]
                rows_pc.append(ns + [-1] * (P - len(ns)))
            xp = []
            for c in range(n_cores):
                xp.append(np.array(rows_pc[c], np.int64))
            xp_idx = len(xp_cols)
            xp_cols.append(xp)
            if l == 0:
                tiles.append(dict(kind="leaf", sti=(base // P) + t,
                                  xp_idx=xp_idx, ets=[], h_pairs=[]))
                continue
            edges_pc = []
            for c in range(n_cores):
                ed = []
                for prow, node in enumerate(rows_pc[c]):
                    if node < 0:
                        continue
                    for ch in children_of(node):
                        ch = int(ch)
                        ed.append((int(slot_of[c][ch]), prow, node))
                ed.sort(key=lambda x: x[0])
                edges_pc.append(ed)
            ets = build_edge_struct(edges_pc)
            h_pairs = build_h_pairs(rows_pc)
            tiles.append(dict(kind="bulk", sti=(base // P) + t,
                              xp_idx=xp_idx, ets=ets, h_pairs=h_pairs))

    # ---- pseudo tiles
    S_tail_pad = S_tail
    for t in range(n_pseudo_tiles):
        rows_pc = []
        scat = []
        edges_pc = []
        for c in range(n_cores):
            pn = pseudo_nodes[c][t * P:(t + 1) * P]
            rows = [j for (j, _) in pn] + [-1] * (P - len(pn))
            rows_pc.append(rows)
            sv = np.full(P, 0, np.int32)
            for r in range(P):
                if r < len(pn):
                    sv[r] = crank[int(pn[r][0])]
                else:
                    sv[r] = C_tail + P + r   # junk tile row
            scat.append(sv)
            ed = []
            for prow, (j, chs) in enumerate(pn):
                for ch in chs:
                    ed.append((int(slot_of[c][ch]), prow, j))
            ed.sort(key=lambda x: x[0])
            edges_pc.append(ed)
        scat_idx = len(scat_cols)
        scat_cols.append(scat)
        ets = build_edge_struct(edges_pc, pseudo=True)
        tiles.append(dict(kind="pseudo", sti=None, xp_idx=None,
                          ets=ets, h_pairs=[], scat_idx=scat_idx))

    # ---- tail levels ascending
    n_tail_tiles = S_tail // P
    for (l, base, nt, ns) in reversed(tail_level_meta):
        for t in range(nt):
            rows = ns[t * P:(t + 1) * P]
            rows = rows + [-1] * (P - len(rows))
            rows_pc = [rows] * n_cores
            xp = [np.array(rows, np.int64)] * n_cores
            xp_idx = len(xp_cols)
            xp_cols.append(xp)
            edges_pc = []
            for c in range(n_cores):
                ed = []
                for prow, node in enumerate(rows):
                    if node < 0:
                        continue
                    for ch in children_of(node):
                        ch = int(ch)
                        if is_tail[ch]:
                            ed.append((int(slot_of[c][ch]), prow, node))
                ed.sort(key=lambda x: x[0])
                edges_pc.append(ed)
            ets = build_edge_struct(edges_pc)
            h_pairs = build_h_pairs(rows_pc)
            sti = (base // P) + t
            tiles.append(dict(kind="tail", sti=sti, xp_idx=xp_idx,
                              ets=ets, h_pairs=h_pairs, rrt=sti))

    pl = Plan()
    pl.N, pl.n_cores = N, n_cores
    pl.level_cut, pl.size_cap = level_cut, size_cap
    pl.S_tail, pl.S_all = S_tail, S_all
    pl.C_tail = C_tail
    pl.tail_rbase = tail_rbase
    pl.n_state_tiles = S_all // P
    pl.n_tail_tiles = n_tail_tiles
    pl.slot_of = slot_of
    pl.core_of = core_of
    pl.is_tail = is_tail
    pl.tiles = tiles
    pl.tables = tables
    pl.xp_cols = xp_cols
    pl.xe_cols = xe_cols
    pl.scat_cols = scat_cols
    pl.n_tables = len(tables[0])
    return pl


# ---------------------------------------------------------------------------
# bass program
# ---------------------------------------------------------------------------

def build_kernel(pl, IN=300, M=150):
    import concourse.bass as bass
    import concourse.bacc as bacc
    import concourse.mybir as mybir
    import concourse.tile as tile
    from concourse.masks import make_identity

    F32 = mybir.dt.float32
    BF16 = mybir.dt.bfloat16
    I32 = mybir.dt.int32
    SIG = mybir.ActivationFunctionType.Sigmoid
    TANH = mybir.ActivationFunctionType.Tanh
    EQ = mybir.AluOpType.is_equal

    n_cores = pl.n_cores
    M3 = 3 * M                      # 450
    ROW = 3 * M                     # state row [fh | c | h]
    TP = len(pl.xp_cols)
    TE = len(pl.xe_cols)
    NPS = max(1, len(pl.scat_cols))
    NT = max(1, pl.n_tables)
    R_rows = ((pl.C_tail + P - 1) // P) * P + 2 * P   # + junk rows

    kchunks = [(0, P), (P, P), (2 * P, KA - 2 * P)]   # within padded 384

    nc = bacc.Bacc("TRN2", target_bir_lowering=False, debug=False,
                   num_devices=n_cores, num_swdge_queues=4)

    NTI = NT + P   # extra 128 iota columns appended host-side

    x_tp = nc.dram_tensor("x_tp", [P, TP * KPAD], BF16, kind="ExternalInput")
    x_te = nc.dram_tensor("x_te", [P, max(1, TE) * KPAD], BF16,
                          kind="ExternalInput")
    tb_d = nc.dram_tensor("tb", [P, NTI], mybir.dt.float32, kind="ExternalInput")
    scat_d = nc.dram_tensor("scat", [P, NPS], I32, kind="ExternalInput")
    w_iou_d = nc.dram_tensor("w_iou", [KA, M3], BF16, kind="ExternalInput")
    w_f_d = nc.dram_tensor("w_f", [KA, M], BF16, kind="ExternalInput")
    w_iouh_d = nc.dram_tensor("w_iouh", [M, M3], BF16, kind="ExternalInput")
    w_fh_d = nc.dram_tensor("w_fh", [M, M], BF16, kind="ExternalInput")
    out_d = nc.dram_tensor("out", [pl.S_all, M], BF16, kind="ExternalOutput")
    partial_d = nc.dram_tensor("partial", [R_rows, 2 * M], BF16, kind="Internal")
    reduced_d = nc.dram_tensor("reduced", [R_rows, 2 * M], BF16,
                               kind="Internal", addr_space="Shared")

    mchunks = [(0, P), (P, M - P)]   # 150 = 128 + 22

    with tile.TileContext(nc) as tc, ExitStack() as ctx:
        const = ctx.enter_context(tc.tile_pool(name="const", bufs=1))
        stp = ctx.enter_context(tc.tile_pool(name="stp", bufs=1))
        xpool = ctx.enter_context(tc.tile_pool(name="xpool", bufs=12))
        selp = ctx.enter_context(tc.tile_pool(name="selp", bufs=10))
        wk = ctx.enter_context(tc.tile_pool(name="wk", bufs=4))
        rsp = ctx.enter_context(tc.tile_pool(name="rsp", bufs=1))
        psA = ctx.enter_context(tc.tile_pool(name="psA", bufs=2, space="PSUM"))
        psE = ctx.enter_context(tc.tile_pool(name="psE", bufs=2, space="PSUM"))
        psM = ctx.enter_context(tc.tile_pool(name="psM", bufs=2, space="PSUM"))
        psT = ctx.enter_context(tc.tile_pool(name="psT", bufs=2, space="PSUM"))

        # ---------- constants ----------
        ident_f = const.tile([P, P], F32)
        make_identity(nc, ident_f[:])
        ident_b = const.tile([P, P], BF16)
        nc.vector.tensor_copy(out=ident_b[:], in_=ident_f[:])

        tb_sb = const.tile([P, NTI], F32)
        nc.sync.dma_start(out=tb_sb[:], in_=tb_d.ap())
        iota_f = tb_sb[:, NT:NTI]
        scat_sb = const.tile([P, NPS], I32)
        nc.sync.dma_start(out=scat_sb[:], in_=scat_d.ap())

        w_iou_sb, w_f_sb = [], []
        for ci, (k0, kn) in enumerate(kchunks):
            t = const.tile([P, M3], BF16, tag=f"wiou{ci}")
            nc.sync.dma_start(out=t[:kn, :], in_=w_iou_d.ap()[k0:k0 + kn, :])
            w_iou_sb.append(t)
            t2 = const.tile([P, M], BF16, tag=f"wf{ci}")
            nc.sync.dma_start(out=t2[:kn, :], in_=w_f_d.ap()[k0:k0 + kn, :])
            w_f_sb.append(t2)
        w_iouh_sb, w_fh_sb = [], []
        for ci, (m0, mn) in enumerate(mchunks):
            t = const.tile([P, M3], BF16, tag=f"wiouh{ci}")
            nc.sync.dma_start(out=t[:mn, :], in_=w_iouh_d.ap()[m0:m0 + mn, :])
            w_iouh_sb.append(t)
            t2 = const.tile([P, M], BF16, tag=f"wfh{ci}")
            nc.sync.dma_start(out=t2[:mn, :], in_=w_fh_d.ap()[m0:m0 + mn, :])
            w_fh_sb.append(t2)

        zero_b = const.tile([P, 2 * M], BF16)
        nc.gpsimd.memset(zero_b[:], 0.0)
        for r0 in range(0, R_rows, P):
            nc.sync.dma_start(out=partial_d.ap()[r0:r0 + P, :], in_=zero_b[:])

        # state tiles
        st = []
        for i in range(pl.n_state_tiles):
            sti_t = stp.tile([P, ROW], BF16, tag=f"st{i}", name=f"st{i}")
            st.append(sti_t)

        # ---------- helpers ----------
        G = 8   # x tiles per grouped DMA
        xg_state = {"p": {}, "e": {}}

        def load_x_group(kindkey, dram, idx, tag, n_idx):
            '''Group consecutive x tiles into one DMA.  Returns (ap, toff)
            such that chunk ci of this tile is ap[:, (ci*gsz+toff)*P : +P].'''
            gid = idx // G
            cache = xg_state[kindkey]
            if gid not in cache:
                g0 = gid * G
                gsz = min(G, n_idx - g0)
                xg = xpool.tile([P, gsz * KPAD], BF16, tag=tag, bufs=3,
                                name=f"{tag}{gid}")
                nc.sync.dma_start(
                    out=xg[:, 0:gsz * KPAD],
                    in_=dram.ap()[:, g0 * KPAD:(g0 + gsz) * KPAD])
                cache[gid] = (xg, g0, gsz)
            xg, g0, gsz = cache[gid]
            return xg, idx - g0, gsz

        def build_ind(col, tag="sel"):
            m = selp.tile([P, P], BF16, tag=tag, bufs=28)
            nc.vector.tensor_scalar(out=m[:], in0=iota_f,
                                    scalar1=tb_sb[:, col:col + 1],
                                    scalar2=None, op0=EQ)
            return m

        def emit_loads(t):
            if t["xp_idx"] is not None:
                t["xt"] = load_x_group("p", x_tp, t["xp_idx"], "xg", TP)
            for et in t["ets"]:
                et["xe"] = load_x_group("e", x_te, et["xe_idx"], "eg",
                                        max(1, TE))

        def emit_compute(t):
            kind = t["kind"]
            pseudo = kind == "pseudo"
            has_e = len(t["ets"]) > 0
            has_hp = len(t["h_pairs"]) > 0
            has_h = has_hp or kind == "tail"

            # ---- x GEMM (iou) ----
            if not pseudo:
                # the iou accumulation group is closed by iouh mms (bulk),
                # reduced-transpose mms (tail), or the last x chunk (leaf)
                x_closes = not has_h and kind != "tail"
                psum_iou = psA.tile([P, M3], F32, tag="iou")
                xg, toff, gsz = t["xt"]
                for ci, (k0, kn) in enumerate(kchunks):
                    c0 = (toff * KC + ci) * P
                    nc.tensor.matmul(out=psum_iou[:],
                                     lhsT=xg[:kn, c0:c0 + P],
                                     rhs=w_iou_sb[ci][:kn, :],
                                     start=(ci == 0),
                                     stop=(x_closes and ci == KC - 1))
            else:
                psum_hs = psA.tile([P, M3], F32, tag="iou")

            # ---- h_sum aggregation (feat-major) ----
            if (has_hp or has_e) and not pseudo:
                psum_misc = psM.tile([P, 408], F32, tag="misc")
            if has_hp and not pseudo:
                np_ = len(t["h_pairs"])
                inccs = []
                for (seg, col) in t["h_pairs"]:
                    incc_m = build_ind(col, "incc")
                    inccs.append(incc_m)
                # two sequential accumulation groups: interleaving two open
                # groups in one PSUM bank corrupts the first contribution
                for pi, (seg, col) in enumerate(t["h_pairs"]):
                    nc.tensor.matmul(out=psum_misc[:, 0:P],
                                     lhsT=st[seg][:, 2 * M:2 * M + P],
                                     rhs=inccs[pi][:],
                                     start=(pi == 0), stop=(pi == np_ - 1))
                for pi, (seg, col) in enumerate(t["h_pairs"]):
                    nc.tensor.matmul(out=psum_misc[:M - P, P:2 * P],
                                     lhsT=st[seg][:, 2 * M + P:3 * M],
                                     rhs=inccs[pi][:],
                                     start=(pi == 0), stop=(pi == np_ - 1))

            # ---- edge phase ----
            if has_e:
                ne = len(t["ets"])
                for ei, et in enumerate(t["ets"]):
                    pe_ = psE.tile([P, M3], F32, tag="pe")
                    wid = M3 if pseudo else 2 * M
                    npair = len(et["pairs"])
                    sels = [build_ind(col, "sel") for (seg, col) in et["pairs"]]
                    # [fh_e | c_e (| h_e)] in one wide mm per pair, then the
                    # fx GEMM accumulates into the [0:M] sub-region
                    swid = 3 * M if pseudo else 2 * M
                    for pi, (seg, col) in enumerate(et["pairs"]):
                        nc.tensor.matmul(out=pe_[:, 0:swid], lhsT=sels[pi][:],
                                         rhs=st[seg][:, 0:swid],
                                         start=(pi == 0), stop=False)
                    exg, etoff, egsz = et["xe"]
                    for ci, (k0, kn) in enumerate(kchunks):
                        c0 = (etoff * KC + ci) * P
                        nc.tensor.matmul(out=pe_[:, 0:M],
                                         lhsT=exg[:kn, c0:c0 + P],
                                         rhs=w_f_sb[ci][:kn, :],
                                         start=False, stop=(ci == KC - 1),
                                         skip_group_check=True)
                    f_sb = wk.tile([P, M], F32, tag="f")
                    nc.scalar.activation(out=f_sb[:], in_=pe_[:, 0:M], func=SIG)
                    inc = build_ind(et["ptab_col"], "inc")
                    if pseudo:
                        hfc = wk.tile([P, 2 * M], BF16, tag="hfc")
                        nc.scalar.copy(out=hfc[:, 0:M],
                                       in_=pe_[:, 2 * M:3 * M])
                        nc.vector.tensor_mul(out=hfc[:, M:2 * M], in0=f_sb[:],
                                             in1=pe_[:, M:2 * M])
                        nc.tensor.matmul(out=psum_hs[:, 0:2 * M], lhsT=inc[:],
                                         rhs=hfc[:],
                                         start=(ei == 0), stop=(ei == ne - 1))
                    else:
                        fc_sb = wk.tile([P, M], BF16, tag="fc")
                        nc.vector.tensor_mul(out=fc_sb[:], in0=f_sb[:],
                                             in1=pe_[:, M:2 * M])
                        nc.tensor.matmul(out=psum_misc[:, 256:256 + M],
                                         lhsT=inc[:], rhs=fc_sb[:],
                                         start=(ei == 0), stop=(ei == ne - 1))

            if pseudo:
                hs_sb = wk.tile([P, 2 * M], BF16, tag="hssc")
                nc.vector.tensor_copy(out=hs_sb[:], in_=psum_hs[:, 0:2 * M])
                nc.gpsimd.indirect_dma_start(
                    out=partial_d.ap(), in_=hs_sb[:],
                    out_offset=bass.IndirectOffsetOnAxis(
                        ap=scat_sb[:, t["scat_idx"]:t["scat_idx"] + 1], axis=0),
                    in_offset=None)
                return

            # ---- iouh GEMM from feat-major h_sum ----
            last_hs = not (kind == "tail")
            if has_hp:
                hsT = wk.tile([P, 2 * P], BF16, tag="hsT")
                nc.scalar.copy(out=hsT[:, 0:P], in_=psum_misc[:, 0:P])
                nc.vector.tensor_copy(out=hsT[:M - P, P:2 * P],
                                      in_=psum_misc[:M - P, P:2 * P])
                nc.tensor.matmul(out=psum_iou[:], lhsT=hsT[:, 0:P],
                                 rhs=w_iouh_sb[0][:P, :],
                                 start=False, stop=False)
                nc.tensor.matmul(out=psum_iou[:], lhsT=hsT[:M - P, P:2 * P],
                                 rhs=w_iouh_sb[1][:M - P, :],
                                 start=False, stop=last_hs)
            rsb = None
            if kind == "tail":
                rsb = t["rsb"]
                trp = psT.tile([P, 2 * P], BF16, tag="tr")
                nc.tensor.transpose(out=trp[:P, 0:P], in_=rsb[:, 0:P],
                                    identity=ident_b[:])
                nc.tensor.transpose(out=trp[:M - P, P:2 * P],
                                    in_=rsb[:, P:M], identity=ident_b[:])
                rsT = wk.tile([P, 2 * P], BF16, tag="rsT")
                nc.vector.tensor_copy(out=rsT[:, 0:P], in_=trp[:, 0:P])
                nc.vector.tensor_copy(out=rsT[:M - P, P:2 * P],
                                      in_=trp[:M - P, P:2 * P])
                nc.tensor.matmul(out=psum_iou[:], lhsT=rsT[:, 0:P],
                                 rhs=w_iouh_sb[0][:P, :],
                                 start=False, stop=False)
                nc.tensor.matmul(out=psum_iou[:], lhsT=rsT[:M - P, P:2 * P],
                                 rhs=w_iouh_sb[1][:M - P, :],
                                 start=False, stop=True)
            # gates
            gio = wk.tile([P, 2 * M], F32, tag="gio")
            nc.scalar.activation(out=gio[:], in_=psum_iou[:, 0:2 * M], func=SIG)
            gu = wk.tile([P, M], F32, tag="gu")
            nc.scalar.activation(out=gu[:], in_=psum_iou[:, 2 * M:3 * M],
                                 func=TANH)
            sti = t["sti"]
            c_dst = st[sti][:, M:2 * M]
            if kind == "leaf":
                nc.vector.tensor_mul(out=c_dst, in0=gio[:, 0:M], in1=gu[:])
            else:
                cf = wk.tile([P, M], F32, tag="cf")
                nc.vector.tensor_mul(out=cf[:], in0=gio[:, 0:M], in1=gu[:])
                if kind == "tail":
                    if has_e:
                        nc.vector.tensor_add(out=cf[:], in0=cf[:],
                                             in1=psum_misc[:, 256:256 + M])
                    nc.vector.tensor_add(out=c_dst, in0=cf[:],
                                         in1=rsb[:, M:2 * M])
                else:
                    nc.vector.tensor_add(out=c_dst, in0=cf[:],
                                         in1=psum_misc[:, 256:256 + M])
            tc_ = wk.tile([P, M], F32, tag="tc")
            nc.scalar.activation(out=tc_[:], in_=c_dst, func=TANH)
            nc.vector.tensor_mul(out=st[sti][:, 2 * M:3 * M],
                                 in0=gio[:, M:2 * M], in1=tc_[:])       # h
            # fh = h @ W_fh.T
            trh = psT.tile([P, 2 * P], BF16, tag="tr")
            nc.tensor.transpose(out=trh[:P, 0:P],
                                in_=st[sti][:, 2 * M:2 * M + P],
                                identity=ident_b[:])
            nc.tensor.transpose(out=trh[:M - P, P:2 * P],
                                in_=st[sti][:, 2 * M + P:3 * M],
                                identity=ident_b[:])
            hT = wk.tile([P, 2 * P], BF16, tag="hT")
            nc.scalar.copy(out=hT[:, 0:P], in_=trh[:, 0:P])
            nc.vector.tensor_copy(out=hT[:M - P, P:2 * P],
                                  in_=trh[:M - P, P:2 * P])
            psum_fh = psE.tile([P, M3], F32, tag="pe")
            nc.tensor.matmul(out=psum_fh[:, 0:M], lhsT=hT[:, 0:P],
                             rhs=w_fh_sb[0][:P, :], start=True, stop=False)
            nc.tensor.matmul(out=psum_fh[:, 0:M], lhsT=hT[:M - P, P:2 * P],
                             rhs=w_fh_sb[1][:M - P, :], start=False, stop=True)
            nc.scalar.copy(out=st[sti][:, 0:M], in_=psum_fh[:, 0:M])
            # output h
            nc.gpsimd.dma_start(out=out_d.ap()[sti * P:(sti + 1) * P, :],
                                in_=st[sti][:, 2 * M:3 * M])

        # ================= main schedule =================
        PF = 8
        tiles = pl.tiles
        n_tiles = len(tiles)
        first_tail = next(i for i, t in enumerate(tiles)
                          if t["kind"] == "tail")
        allreduce_done = [False]

        def maybe_allreduce(i):
            if allreduce_done[0]:
                return
            allreduce_done[0] = True
            nc.gpsimd.collective_compute(
                "AllReduce", mybir.AluOpType.add,
                replica_groups=[list(range(n_cores))],
                ins=[partial_d.ap()], outs=[reduced_d.ap()])
            # load reduced into SBUF tiles
            for rt in range(pl.n_tail_tiles):
                rsb = rsp.tile([P, 2 * M], BF16, tag=f"rsb{rt}")
                rb = pl.tail_rbase[rt]
                nc.sync.dma_start(out=rsb[:],
                                  in_=reduced_d.ap()[rb:rb + P, :])
                for tt in tiles:
                    if tt["kind"] == "tail" and tt["rrt"] == rt:
                        tt["rsb"] = rsb

        for i in range(n_tiles + PF):
            if i < n_tiles:
                emit_loads(tiles[i])
            j = i - PF
            if j >= 0:
                if j == first_tail:
                    maybe_allreduce(j)
                emit_compute(tiles[j])

    nc.compile()
    return nc


# ---------------------------------------------------------------------------
# host-side input packing / output assembly
# ---------------------------------------------------------------------------

def build_inputs(pl, x, W_ioux, b_ioux, W_iouh, b_iouh, W_fx, b_fx, W_fh,
                 b_fh):
    import ml_dtypes
    bf16 = ml_dtypes.bfloat16
    N, IN = x.shape
    M = W_fh.shape[0]
    n_cores = pl.n_cores
    TP = len(pl.xp_cols)
    TE = len(pl.xe_cols)

    xa = np.concatenate([x.astype(np.float32),
                         np.ones((N, 1), np.float32)], axis=1)   # [N, 301]
    xa_b = xa.astype(bf16)

    w_iou = np.zeros((KA, 3 * M), np.float32)
    w_iou[:IN] = W_ioux.T
    w_iou[IN] = b_ioux + b_iouh
    w_f = np.zeros((KA, M), np.float32)
    w_f[:IN] = W_fx.T
    w_f[IN] = b_fx + b_fh
    w_iouh = np.ascontiguousarray(W_iouh.T)
    w_fh = np.ascontiguousarray(W_fh.T)

    def pack_x(cols_list):
        # cols_list: list over tiles of per-core [128] node ids.
        # Output layout: [128 partitions, T*3*128] where partition p holds
        # feature rows {p, 128+p, 256+p} of each tile (chunk-major inside
        # tile) -> one contiguous 2D DMA per tile group.
        T = len(cols_list)
        out = []
        for c in range(n_cores):
            arr = np.zeros((T, KC, P, P), np.float32)   # [t, ci, p, w]
            for ti, cols in enumerate(cols_list):
                ids = cols[c]
                sel = ids >= 0
                if sel.any():
                    block = np.zeros((KPAD, P), np.float32)
                    block[:KA, sel] = xa[ids[sel]].T
                    arr[ti] = block.reshape(KC, P, P)
            out.append(np.ascontiguousarray(
                arr.transpose(2, 0, 1, 3).reshape(P, T * KPAD)).astype(bf16))
        return out

    x_tp_pc = pack_x(pl.xp_cols)
    if TE:
        x_te_pc = pack_x(pl.xe_cols)
    else:
        x_te_pc = [np.zeros((P, KPAD), bf16) for _ in range(n_cores)]

    NT = max(1, pl.n_tables)
    iota_cols = np.tile(np.arange(P, dtype=np.float32)[None, :], (P, 1))
    tb_pc = []
    for c in range(n_cores):
        tbl = np.full((P, NT + P), -1.0, np.float32)
        for col in range(pl.n_tables):
            tbl[:, col] = pl.tables[c][col]
        tbl[:, NT:NT + P] = iota_cols
        tb_pc.append(tbl)

    NPS = max(1, len(pl.scat_cols))
    scat_pc = []
    for c in range(n_cores):
        sc = np.zeros((P, NPS), np.int32)
        sc[:, :] = pl.C_tail + P + np.arange(P)[:, None]  # junk default
        for col, scat in enumerate(pl.scat_cols):
            sc[:, col] = scat[c]
        scat_pc.append(sc)

    in_maps = []
    for c in range(n_cores):
        in_maps.append({
            "x_tp": x_tp_pc[c],
            "x_te": x_te_pc[c],
            "tb": tb_pc[c],
            "scat": scat_pc[c],
            "w_iou": w_iou.astype(bf16),
            "w_f": w_f.astype(bf16),
            "w_iouh": w_iouh.astype(bf16),
            "w_fh": w_fh.astype(bf16),
        })
    return in_maps


def assemble_output(pl, results, M=150):
    N = pl.N
    h_full = np.zeros((N, M), np.float32)
    for c in range(pl.n_cores):
        nodes = np.nonzero(pl.core_of[:N] == c)[0]
        if len(nodes):
            slots = pl.slot_of[c][nodes]
            h_full[nodes] = results[c]["out"][slots].astype(np.float32)
    tnodes = np.nonzero(pl.is_tail)[0]
    if len(tnodes):
        slots = pl.slot_of[0][tnodes]
        h_full[tnodes] = results[0]["out"][slots].astype(np.float32)
    return h_full


_PROFILE_STATE = {"exec_ns": None}


def _install_profile_hook():
    import sys, types
    try:
        import antenv.axon_hooks  # noqa: F401
        return True
    except ImportError:
        pass
    try:
        import antenv
        from trn_agent_boot.trn_boot import _ntff_profile_via_ctypes
    except ImportError:
        return False
    hook = _ntff_profile_via_ctypes("/opt/axon/libaxon_pjrt.so")
    if hook is None:
        return False
    mod = types.ModuleType("antenv.axon_hooks")
    state = {"h": hook}
    mod.set_axon_ntff_profile_hook = lambda h: state.__setitem__("h", h)
    mod.get_axon_ntff_profile_hook = lambda: state["h"]
    sys.modules["antenv.axon_hooks"] = mod
    antenv.axon_hooks = mod
    return True


def kernel(x, W_ioux, b_ioux, W_iouh, b_iouh, W_fx, b_fx, W_fh, b_fh, parents):
    import os
    from concourse import bass_utils

    x = np.asarray(x, np.float32)
    parents_np = np.asarray(parents).astype(np.int64)
    pl = build_plan(parents_np, n_cores=8)
    nc = build_kernel(pl, IN=x.shape[1], M=np.asarray(W_fh).shape[0])
    in_maps = build_inputs(pl, x,
                           np.asarray(W_ioux, np.float32),
                           np.asarray(b_ioux, np.float32),
                           np.asarray(W_iouh, np.float32),
                           np.asarray(b_iouh, np.float32),
                           np.asarray(W_fx, np.float32),
                           np.asarray(b_fx, np.float32),
                           np.asarray(W_fh, np.float32),
                           np.asarray(b_fh, np.float32))
    trace = os.environ.get("TREELSTM_PROFILE", "") == "1"
    if trace:
        trace = _install_profile_hook()
    res = bass_utils.run_bass_kernel_spmd(
        nc, in_maps, core_ids=list(range(8)), trace=trace)
    _PROFILE_STATE["exec_ns"] = res.exec_time_ns
    return assemble_output(pl, res.results).astype(np.float32)


# revision 29
# speedup vs baseline: 1.1267x; 1.1267x over previous
"""Self-contained ChildSum TreeLSTM kernel for 8 Trainium2 NeuronCores.

Strategy v2: subtree-partitioned data parallelism with fully SBUF-resident
node states and matmul-based child aggregation.

  - Bulk nodes (small subtrees) are packed into 8 per-core forests; the top
    "tail" nodes (level >= cut or huge subtree) are replicated on every core.
    Each core computes partial child-sums for tail nodes from its own bulk
    children (pseudo phase), one AllReduce combines them, then every core
    runs the tail levels identically.
  - Node states [fh | c | h] (fh = h @ W_fh.T) live in SBUF for the whole
    kernel (bf16).  Child gathers/scatters are 0/1 incidence matmuls; the
    incidence matrices are built on-device with one tensor_scalar(is_equal)
    op each from tiny per-tile index tables.
  - Slots are assigned top-down sorted by parent slot, so each parent tile's
    children form few contiguous runs -> few (edge-tile, segment) pairs.
  - All GEMMs in bf16 with f32 PSUM accumulation.

kernel(**inputs) takes full unsharded inputs, returns full [N, 150] f32 h.
"""
from contextlib import ExitStack

import numpy as np

P = 128
KC = 3          # K chunks for the 301-row contraction (padded to 384)
KA = 301        # 300 features + bias row
KPAD = KC * P   # 384


class Plan:
    pass


def _tiles(n):
    return (n + P - 1) // P


# ---------------------------------------------------------------------------
# schedule / partition
# ---------------------------------------------------------------------------

def build_plan(parents, n_cores=8):
    parents = np.asarray(parents, dtype=np.int64)
    N = parents.shape[0]

    level = np.zeros(N, np.int64)
    size = np.ones(N, np.int64)
    for i in range(N - 1):
        p = parents[i]
        if level[i] + 1 > level[p]:
            level[p] = level[i] + 1
        size[p] += size[i]
    height = int(level.max())

    # children CSR (children of j sorted ascending)
    order_by_parent = np.argsort(parents[: N - 1], kind="stable")
    csr_off = np.zeros(N + 2, np.int64)
    np.add.at(csr_off, parents[: N - 1] + 1, 1)
    csr_off = np.cumsum(csr_off)
    csr_children = order_by_parent

    def children_of(j):
        return csr_children[csr_off[j]: csr_off[j + 1]]

    # --- tail cut
    counts = np.bincount(level, minlength=height + 1)
    level_cut = height + 1
    for l in range(height + 1):
        if counts[l] <= n_cores * 40:
            level_cut = l
            break
    size_cap = max(256, int(N // (n_cores * 1.35)))
    is_tail = (level >= level_cut) | (size > size_cap)
    tail_nodes = np.nonzero(is_tail)[0]
    bulk_mask = ~is_tail

    # --- subtree roots -> LPT binning
    sub_roots = [i for i in np.nonzero(bulk_mask)[0]
                 if parents[i] == N or is_tail[parents[i]]]
    root_of = np.full(N, -1, np.int64)
    for r in sub_roots:
        root_of[r] = r
    for i in range(N - 2, -1, -1):
        if bulk_mask[i] and root_of[i] == -1:
            root_of[i] = root_of[parents[i]]
    sub_roots_arr = np.array(sub_roots, np.int64)
    order = np.argsort(-size[sub_roots_arr], kind="stable")
    bin_tot = np.zeros(n_cores, np.int64)
    bin_of_root = {}
    for k in order:
        b = int(np.argmin(bin_tot))
        bin_of_root[int(sub_roots_arr[k])] = b
        bin_tot[b] += size[sub_roots_arr[k]]
    core_of = np.full(N, -1, np.int8)
    bm = np.nonzero(bulk_mask)[0]
    core_of[bm] = [bin_of_root[int(root_of[i])] for i in bm]

    # --- slot assignment, levels descending (tail levels first, then bulk)
    tail_levels = sorted({int(level[i]) for i in tail_nodes}, reverse=True)
    bulk_levels_desc = list(range(level_cut - 1, -1, -1))

    slot_of = [np.full(N, -1, np.int64) for _ in range(n_cores)]
    # big offset so bulk parents sort after tail parents
    BIG = 1 << 40

    def parent_key(c, i):
        p = parents[i]
        if p >= N:
            return -1
        if is_tail[p]:
            return slot_of[0][p]          # tail slots shared across cores
        return BIG + slot_of[c][p]

    tail_tile_rows = []   # list of (level, [node rows]) per tail tile, ascending later
    cur = 0
    tail_level_meta = []  # (level, base_slot, n_tiles, rows list)
    for l in tail_levels:
        ns = sorted(int(i) for i in tail_nodes if level[i] == l)
        ns.sort(key=lambda i: parent_key(0, i))
        for r, i in enumerate(ns):
            for c in range(n_cores):
                slot_of[c][i] = cur + r
        nt = _tiles(len(ns))
        tail_level_meta.append((l, cur, nt, ns))
        cur += nt * P
    S_tail = cur
    # compact rank (by slot order) for the partial/reduced buffers
    crank = {}
    tail_rbase = []
    cr = 0
    for (l, base, nt, ns) in tail_level_meta:
        for t in range(nt):
            tail_rbase.append(cr)
            for i in ns[t * P:(t + 1) * P]:
                crank[int(i)] = cr
                cr += 1
    C_tail = cr

    bulk_level_meta = []  # (level, base_slot, n_tiles, rows per core)
    core_level_nodes = {}
    for l in bulk_levels_desc:
        percore = []
        mx = 0
        for c in range(n_cores):
            ns = [int(i) for i in bm if core_of[i] == c and level[i] == l]
            ns.sort(key=lambda i: parent_key(c, i))
            percore.append(ns)
            mx = max(mx, len(ns))
        if mx == 0:
            continue
        for c in range(n_cores):
            for r, i in enumerate(percore[c]):
                slot_of[c][i] = cur + r
        nt = _tiles(mx)
        bulk_level_meta.append((l, cur, nt, percore))
        cur += nt * P
    S_all = cur

    # --- pseudo parents (boundary): tail nodes with bulk children in core c
    bnd = [dict() for _ in range(n_cores)]
    for i in bm:
        p = int(parents[i])
        if p < N and is_tail[p]:
            bnd[core_of[i]].setdefault(p, []).append(int(i))
    pseudo_nodes = []
    for c in range(n_cores):
        ks = sorted(bnd[c].keys(), key=lambda j: slot_of[0][j])
        pseudo_nodes.append([(j, sorted(bnd[c][j])) for j in ks])
    n_pseudo_tiles = max((_tiles(len(pn)) for pn in pseudo_nodes), default=0)

    # ------------------------------------------------------------------
    # tile stream construction (compute order: bulk asc, pseudo, tail asc)
    # ------------------------------------------------------------------
    tables = [[] for _ in range(n_cores)]   # list of [128] f32 arrays per core

    def alloc_col(vals_per_core):
        col = len(tables[0])
        for c in range(n_cores):
            tables[c].append(vals_per_core[c])
        return col

    xp_cols = []      # per x_tp tile: per-core [128] node ids (-1 absent)
    xe_cols = []      # per x_te tile: per-core [128] node ids (parent of edge)
    scat_cols = []    # per pseudo tile: per-core [128] int32 partial rows

    tiles = []

    def build_edge_struct(edges_pc, pseudo=False):
        """edges_pc: per-core list of (child_slot, parent_row, parent_node).
        Returns list of ets."""
        net = max((_tiles(len(e)) for e in edges_pc), default=0)
        ets = []
        for k in range(net):
            chunk = [e[k * P:(k + 1) * P] for e in edges_pc]
            segs = sorted({cs // P for c in range(n_cores)
                           for (cs, _, _) in chunk[c]})
            pairs = []
            for seg in segs:
                vals = []
                for c in range(n_cores):
                    v = np.full(P, -1.0, np.float32)
                    for e, (cs, _, _) in enumerate(chunk[c]):
                        if cs // P == seg:
                            v[cs % P] = e
                    vals.append(v)
                pairs.append((seg, alloc_col(vals)))
            pt = []
            xe = []
            for c in range(n_cores):
                v = np.full(P, -1.0, np.float32)
                xv = np.full(P, -1, np.int64)
                for e, (cs, prow, pnode) in enumerate(chunk[c]):
                    v[e] = prow
                    xv[e] = pnode
                pt.append(v)
                xe.append(xv)
            ptab_col = alloc_col(pt)
            xe_idx = len(xe_cols)
            xe_cols.append(xe)
            ets.append(dict(xe_idx=xe_idx, ptab_col=ptab_col, pairs=pairs))
        return ets

    def build_h_pairs(rows_pc):
        """rows_pc: per-core list of node ids (tile rows). h_pairs over all
        children (child_slot -> parent_row)."""
        allpairs = {}
        for c in range(n_cores):
            for prow, node in enumerate(rows_pc[c]):
                if node < 0:
                    continue
                for ch in children_of(node):
                    ch = int(ch)
                    if is_tail[node] and not is_tail[ch]:
                        continue  # bulk children of tail handled via pseudo
                    seg = int(slot_of[c][ch]) // P
                    allpairs.setdefault(seg, None)
        h_pairs = []
        for seg in sorted(allpairs.keys()):
            vals = []
            for c in range(n_cores):
                v = np.full(P, -1.0, np.float32)
                for prow, node in enumerate(rows_pc[c]):
                    if node < 0:
                        continue
                    for ch in children_of(node):
                        ch = int(ch)
                        if is_tail[node] and not is_tail[ch]:
                            continue
                        s = int(slot_of[c][ch])
                        if s // P == seg:
                            v[s % P] = prow
                vals.append(v)
            h_pairs.append((seg, alloc_col(vals)))
        return h_pairs

    # ---- bulk levels ascending
    for (l, base, nt, percore) in reversed(bulk_level_meta):
        for t in range(nt):
            rows_pc = []
            for c in range(n_cores):
                ns = percore[c][t * P:(t + 1) * P]
                rows_pc.append(ns + [-1] * (P - len(ns)))
            xp = []
            for c in range(n_cores):
                xp.append(np.array(rows_pc[c], np.int64))
            xp_idx = len(xp_cols)
            xp_cols.append(xp)
            if l == 0:
                tiles.append(dict(kind="leaf", sti=(base // P) + t,
                                  xp_idx=xp_idx, ets=[], h_pairs=[]))
                continue
            edges_pc = []
            for c in range(n_cores):
                ed = []
                for prow, node in enumerate(rows_pc[c]):
                    if node < 0:
                        continue
                    for ch in children_of(node):
                        ch = int(ch)
                        ed.append((int(slot_of[c][ch]), prow, node))
                ed.sort(key=lambda x: x[0])
                edges_pc.append(ed)
            ets = build_edge_struct(edges_pc)
            h_pairs = build_h_pairs(rows_pc)
            tiles.append(dict(kind="bulk", sti=(base // P) + t,
                              xp_idx=xp_idx, ets=ets, h_pairs=h_pairs))

    # ---- pseudo tiles
    S_tail_pad = S_tail
    for t in range(n_pseudo_tiles):
        rows_pc = []
        scat = []
        edges_pc = []
        for c in range(n_cores):
            pn = pseudo_nodes[c][t * P:(t + 1) * P]
            rows = [j for (j, _) in pn] + [-1] * (P - len(pn))
            rows_pc.append(rows)
            sv = np.full(P, 0, np.int32)
            for r in range(P):
                if r < len(pn):
                    sv[r] = crank[int(pn[r][0])]
                else:
                    sv[r] = C_tail + P + r   # junk tile row
            scat.append(sv)
            ed = []
            for prow, (j, chs) in enumerate(pn):
                for ch in chs:
                    ed.append((int(slot_of[c][ch]), prow, j))
            ed.sort(key=lambda x: x[0])
            edges_pc.append(ed)
        scat_idx = len(scat_cols)
        scat_cols.append(scat)
        ets = build_edge_struct(edges_pc, pseudo=True)
        tiles.append(dict(kind="pseudo", sti=None, xp_idx=None,
                          ets=ets, h_pairs=[], scat_idx=scat_idx))

    # ---- tail levels ascending
    n_tail_tiles = S_tail // P
    for (l, base, nt, ns) in reversed(tail_level_meta):
        for t in range(nt):
            rows = ns[t * P:(t + 1) * P]
            rows = rows + [-1] * (P - len(rows))
            rows_pc = [rows] * n_cores
            xp = [np.array(rows, np.int64)] * n_cores
            xp_idx = len(xp_cols)
            xp_cols.append(xp)
            edges_pc = []
            for c in range(n_cores):
                ed = []
                for prow, node in enumerate(rows):
                    if node < 0:
                        continue
                    for ch in children_of(node):
                        ch = int(ch)
                        if is_tail[ch]:
                            ed.append((int(slot_of[c][ch]), prow, node))
                ed.sort(key=lambda x: x[0])
                edges_pc.append(ed)
            ets = build_edge_struct(edges_pc)
            h_pairs = build_h_pairs(rows_pc)
            sti = (base // P) + t
            tiles.append(dict(kind="tail", sti=sti, xp_idx=xp_idx,
                              ets=ets, h_pairs=h_pairs, rrt=sti))

    pl = Plan()
    pl.N, pl.n_cores = N, n_cores
    pl.level_cut, pl.size_cap = level_cut, size_cap
    pl.S_tail, pl.S_all = S_tail, S_all
    pl.C_tail = C_tail
    pl.tail_rbase = tail_rbase
    pl.n_state_tiles = S_all // P
    pl.n_tail_tiles = n_tail_tiles
    pl.slot_of = slot_of
    pl.core_of = core_of
    pl.is_tail = is_tail
    pl.tiles = tiles
    pl.tables = tables
    pl.xp_cols = xp_cols
    pl.xe_cols = xe_cols
    pl.scat_cols = scat_cols
    pl.n_tables = len(tables[0])
    return pl


# ---------------------------------------------------------------------------
# bass program
# ---------------------------------------------------------------------------

def build_kernel(pl, IN=300, M=150):
    import concourse.bass as bass
    import concourse.bacc as bacc
    import concourse.mybir as mybir
    import concourse.tile as tile
    from concourse.masks import make_identity

    F32 = mybir.dt.float32
    BF16 = mybir.dt.bfloat16
    I32 = mybir.dt.int32
    SIG = mybir.ActivationFunctionType.Sigmoid
    TANH = mybir.ActivationFunctionType.Tanh
    EQ = mybir.AluOpType.is_equal

    n_cores = pl.n_cores
    M3 = 3 * M                      # 450
    ROW = 3 * M                     # state row [fh | c | h]
    TP = len(pl.xp_cols)
    TE = len(pl.xe_cols)
    NPS = max(1, len(pl.scat_cols))
    NT = max(1, pl.n_tables)
    R_rows = ((pl.C_tail + P - 1) // P) * P + 2 * P   # + junk rows

    kchunks = [(0, P), (P, P), (2 * P, KA - 2 * P)]   # within padded 384

    nc = bacc.Bacc("TRN2", target_bir_lowering=False, debug=False,
                   num_devices=n_cores, num_swdge_queues=4)

    NTI = NT + P   # extra 128 iota columns appended host-side

    x_tp = nc.dram_tensor("x_tp", [P, TP * KPAD], BF16, kind="ExternalInput")
    x_te = nc.dram_tensor("x_te", [P, max(1, TE) * KPAD], BF16,
                          kind="ExternalInput")
    tb_d = nc.dram_tensor("tb", [P, NTI], mybir.dt.float32, kind="ExternalInput")
    scat_d = nc.dram_tensor("scat", [P, NPS], I32, kind="ExternalInput")
    w_iou_d = nc.dram_tensor("w_iou", [KA, M3], BF16, kind="ExternalInput")
    w_f_d = nc.dram_tensor("w_f", [KA, M], BF16, kind="ExternalInput")
    w_iouh_d = nc.dram_tensor("w_iouh", [M, M3], BF16, kind="ExternalInput")
    w_fh_d = nc.dram_tensor("w_fh", [M, M], BF16, kind="ExternalInput")
    out_d = nc.dram_tensor("out", [pl.S_all, M], BF16, kind="ExternalOutput")
    partial_d = nc.dram_tensor("partial", [R_rows, 2 * M], BF16, kind="Internal")
    reduced_d = nc.dram_tensor("reduced", [R_rows, 2 * M], BF16,
                               kind="Internal", addr_space="Shared")

    mchunks = [(0, P), (P, M - P)]   # 150 = 128 + 22

    with tile.TileContext(nc) as tc, ExitStack() as ctx:
        const = ctx.enter_context(tc.tile_pool(name="const", bufs=1))
        stp = ctx.enter_context(tc.tile_pool(name="stp", bufs=1))
        xpool = ctx.enter_context(tc.tile_pool(name="xpool", bufs=12))
        selp = ctx.enter_context(tc.tile_pool(name="selp", bufs=10))
        wk = ctx.enter_context(tc.tile_pool(name="wk", bufs=4))
        rsp = ctx.enter_context(tc.tile_pool(name="rsp", bufs=1))
        psA = ctx.enter_context(tc.tile_pool(name="psA", bufs=2, space="PSUM"))
        psE = ctx.enter_context(tc.tile_pool(name="psE", bufs=2, space="PSUM"))
        psM = ctx.enter_context(tc.tile_pool(name="psM", bufs=2, space="PSUM"))
        psT = ctx.enter_context(tc.tile_pool(name="psT", bufs=2, space="PSUM"))

        # ---------- constants ----------
        ident_f = const.tile([P, P], F32)
        make_identity(nc, ident_f[:])
        ident_b = const.tile([P, P], BF16)
        nc.vector.tensor_copy(out=ident_b[:], in_=ident_f[:])

        tb_sb = const.tile([P, NTI], F32)
        nc.sync.dma_start(out=tb_sb[:], in_=tb_d.ap())
        iota_f = tb_sb[:, NT:NTI]
        scat_sb = const.tile([P, NPS], I32)
        nc.sync.dma_start(out=scat_sb[:], in_=scat_d.ap())

        w_iou_sb, w_f_sb = [], []
        for ci, (k0, kn) in enumerate(kchunks):
            t = const.tile([P, M3], BF16, tag=f"wiou{ci}")
            nc.sync.dma_start(out=t[:kn, :], in_=w_iou_d.ap()[k0:k0 + kn, :])
            w_iou_sb.append(t)
            t2 = const.tile([P, M], BF16, tag=f"wf{ci}")
            nc.sync.dma_start(out=t2[:kn, :], in_=w_f_d.ap()[k0:k0 + kn, :])
            w_f_sb.append(t2)
        w_iouh_sb, w_fh_sb = [], []
        for ci, (m0, mn) in enumerate(mchunks):
            t = const.tile([P, M3], BF16, tag=f"wiouh{ci}")
            nc.sync.dma_start(out=t[:mn, :], in_=w_iouh_d.ap()[m0:m0 + mn, :])
            w_iouh_sb.append(t)
            t2 = const.tile([P, M], BF16, tag=f"wfh{ci}")
            nc.sync.dma_start(out=t2[:mn, :], in_=w_fh_d.ap()[m0:m0 + mn, :])
            w_fh_sb.append(t2)

        zero_b = const.tile([P, 2 * M], BF16)
        nc.gpsimd.memset(zero_b[:], 0.0)
        for r0 in range(0, R_rows, P):
            nc.sync.dma_start(out=partial_d.ap()[r0:r0 + P, :], in_=zero_b[:])

        # state tiles
        st = []
        for i in range(pl.n_state_tiles):
            sti_t = stp.tile([P, ROW], BF16, tag=f"st{i}", name=f"st{i}")
            st.append(sti_t)

        # ---------- helpers ----------
        G = 8   # x tiles per grouped DMA
        xg_state = {"p": {}, "e": {}}

        def load_x_group(kindkey, dram, idx, tag, n_idx):
            '''Group consecutive x tiles into one DMA.  Returns (ap, toff)
            such that chunk ci of this tile is ap[:, (ci*gsz+toff)*P : +P].'''
            gid = idx // G
            cache = xg_state[kindkey]
            if gid not in cache:
                g0 = gid * G
                gsz = min(G, n_idx - g0)
                xg = xpool.tile([P, gsz * KPAD], BF16, tag=tag, bufs=3,
                                name=f"{tag}{gid}")
                nc.sync.dma_start(
                    out=xg[:, 0:gsz * KPAD],
                    in_=dram.ap()[:, g0 * KPAD:(g0 + gsz) * KPAD])
                cache[gid] = (xg, g0, gsz)
            xg, g0, gsz = cache[gid]
            return xg, idx - g0, gsz

        def build_ind(col, tag="sel"):
            m = selp.tile([P, P], BF16, tag=tag, bufs=28)
            nc.vector.tensor_scalar(out=m[:], in0=iota_f,
                                    scalar1=tb_sb[:, col:col + 1],
                                    scalar2=None, op0=EQ)
            return m

        def emit_loads(t):
            if t["xp_idx"] is not None:
                t["xt"] = load_x_group("p", x_tp, t["xp_idx"], "xg", TP)
            for et in t["ets"]:
                et["xe"] = load_x_group("e", x_te, et["xe_idx"], "eg",
                                        max(1, TE))

        def emit_compute(t):
            kind = t["kind"]
            pseudo = kind == "pseudo"
            has_e = len(t["ets"]) > 0
            has_hp = len(t["h_pairs"]) > 0
            has_h = has_hp or kind == "tail"

            # ---- x GEMM (iou) ----
            if not pseudo:
                # the iou accumulation group is closed by iouh mms (bulk),
                # reduced-transpose mms (tail), or the last x chunk (leaf)
                x_closes = not has_h and kind != "tail"
                psum_iou = psA.tile([P, M3], F32, tag="iou")
                xg, toff, gsz = t["xt"]
                for ci, (k0, kn) in enumerate(kchunks):
                    c0 = (toff * KC + ci) * P
                    nc.tensor.matmul(out=psum_iou[:],
                                     lhsT=xg[:kn, c0:c0 + P],
                                     rhs=w_iou_sb[ci][:kn, :],
                                     start=(ci == 0),
                                     stop=(x_closes and ci == KC - 1))
            else:
                psum_hs = psA.tile([P, M3], F32, tag="iou")

            # ---- h_sum aggregation (feat-major) ----
            if (has_hp or has_e) and not pseudo:
                psum_misc = psM.tile([P, 408], F32, tag="misc")
            if has_hp and not pseudo:
                np_ = len(t["h_pairs"])
                inccs = []
                for (seg, col) in t["h_pairs"]:
                    incc_m = build_ind(col, "incc")
                    inccs.append(incc_m)
                # two sequential accumulation groups: interleaving two open
                # groups in one PSUM bank corrupts the first contribution
                for pi, (seg, col) in enumerate(t["h_pairs"]):
                    nc.tensor.matmul(out=psum_misc[:, 0:P],
                                     lhsT=st[seg][:, 2 * M:2 * M + P],
                                     rhs=inccs[pi][:],
                                     start=(pi == 0), stop=(pi == np_ - 1))
                for pi, (seg, col) in enumerate(t["h_pairs"]):
                    nc.tensor.matmul(out=psum_misc[:M - P, P:2 * P],
                                     lhsT=st[seg][:, 2 * M + P:3 * M],
                                     rhs=inccs[pi][:],
                                     start=(pi == 0), stop=(pi == np_ - 1))

            # ---- edge phase ----
            if has_e:
                ne = len(t["ets"])
                for ei, et in enumerate(t["ets"]):
                    pe_ = psE.tile([P, M3], F32, tag="pe")
                    wid = M3 if pseudo else 2 * M
                    npair = len(et["pairs"])
                    sels = [build_ind(col, "sel") for (seg, col) in et["pairs"]]
                    # [fh_e | c_e (| h_e)] in one wide mm per pair, then the
                    # fx GEMM accumulates into the [0:M] sub-region
                    swid = 3 * M if pseudo else 2 * M
                    for pi, (seg, col) in enumerate(et["pairs"]):
                        nc.tensor.matmul(out=pe_[:, 0:swid], lhsT=sels[pi][:],
                                         rhs=st[seg][:, 0:swid],
                                         start=(pi == 0), stop=False)
                    exg, etoff, egsz = et["xe"]
                    for ci, (k0, kn) in enumerate(kchunks):
                        c0 = (etoff * KC + ci) * P
                        nc.tensor.matmul(out=pe_[:, 0:M],
                                         lhsT=exg[:kn, c0:c0 + P],
                                         rhs=w_f_sb[ci][:kn, :],
                                         start=False, stop=(ci == KC - 1),
                                         skip_group_check=True)
                    f_sb = wk.tile([P, M], F32, tag="f")
                    nc.scalar.activation(out=f_sb[:], in_=pe_[:, 0:M], func=SIG)
                    inc = build_ind(et["ptab_col"], "inc")
                    if pseudo:
                        hfc = wk.tile([P, 2 * M], BF16, tag="hfc")
                        nc.scalar.copy(out=hfc[:, 0:M],
                                       in_=pe_[:, 2 * M:3 * M])
                        nc.vector.tensor_mul(out=hfc[:, M:2 * M], in0=f_sb[:],
                                             in1=pe_[:, M:2 * M])
                        nc.tensor.matmul(out=psum_hs[:, 0:2 * M], lhsT=inc[:],
                                         rhs=hfc[:],
                                         start=(ei == 0), stop=(ei == ne - 1))
                    else:
                        fc_sb = wk.tile([P, M], BF16, tag="fc")
                        nc.vector.tensor_mul(out=fc_sb[:], in0=f_sb[:],
                                             in1=pe_[:, M:2 * M])
                        nc.tensor.matmul(out=psum_misc[:, 256:256 + M],
                                         lhsT=inc[:], rhs=fc_sb[:],
                                         start=(ei == 0), stop=(ei == ne - 1))

            if pseudo:
                hs_sb = wk.tile([P, 2 * M], BF16, tag="hssc")
                nc.vector.tensor_copy(out=hs_sb[:], in_=psum_hs[:, 0:2 * M])
                nc.gpsimd.indirect_dma_start(
                    out=partial_d.ap(), in_=hs_sb[:],
                    out_offset=bass.IndirectOffsetOnAxis(
                        ap=scat_sb[:, t["scat_idx"]:t["scat_idx"] + 1], axis=0),
                    in_offset=None)
                return

            # ---- iouh GEMM from feat-major h_sum ----
            last_hs = not (kind == "tail")
            if has_hp:
                hsT = wk.tile([P, 2 * P], BF16, tag="hsT")
                nc.scalar.copy(out=hsT[:, 0:P], in_=psum_misc[:, 0:P])
                nc.scalar.copy(out=hsT[:M - P, P:2 * P],
                               in_=psum_misc[:M - P, P:2 * P])
                nc.tensor.matmul(out=psum_iou[:], lhsT=hsT[:, 0:P],
                                 rhs=w_iouh_sb[0][:P, :],
                                 start=False, stop=False)
                nc.tensor.matmul(out=psum_iou[:], lhsT=hsT[:M - P, P:2 * P],
                                 rhs=w_iouh_sb[1][:M - P, :],
                                 start=False, stop=last_hs)
            rsb = None
            if kind == "tail":
                rsb = t["rsb"]
                trp = psT.tile([P, 2 * P], BF16, tag="tr")
                nc.tensor.transpose(out=trp[:P, 0:P], in_=rsb[:, 0:P],
                                    identity=ident_b[:])
                nc.tensor.transpose(out=trp[:M - P, P:2 * P],
                                    in_=rsb[:, P:M], identity=ident_b[:])
                rsT = wk.tile([P, 2 * P], BF16, tag="rsT")
                nc.vector.tensor_copy(out=rsT[:, 0:P], in_=trp[:, 0:P])
                nc.vector.tensor_copy(out=rsT[:M - P, P:2 * P],
                                      in_=trp[:M - P, P:2 * P])
                nc.tensor.matmul(out=psum_iou[:], lhsT=rsT[:, 0:P],
                                 rhs=w_iouh_sb[0][:P, :],
                                 start=False, stop=False)
                nc.tensor.matmul(out=psum_iou[:], lhsT=rsT[:M - P, P:2 * P],
                                 rhs=w_iouh_sb[1][:M - P, :],
                                 start=False, stop=True)
            # gates
            gio = wk.tile([P, 2 * M], F32, tag="gio")
            nc.scalar.activation(out=gio[:], in_=psum_iou[:, 0:2 * M], func=SIG)
            gu = wk.tile([P, M], F32, tag="gu")
            nc.scalar.activation(out=gu[:], in_=psum_iou[:, 2 * M:3 * M],
                                 func=TANH)
            sti = t["sti"]
            c_dst = st[sti][:, M:2 * M]
            if kind == "leaf":
                nc.vector.tensor_mul(out=c_dst, in0=gio[:, 0:M], in1=gu[:])
            else:
                cf = wk.tile([P, M], F32, tag="cf")
                nc.vector.tensor_mul(out=cf[:], in0=gio[:, 0:M], in1=gu[:])
                if kind == "tail":
                    if has_e:
                        nc.vector.tensor_add(out=cf[:], in0=cf[:],
                                             in1=psum_misc[:, 256:256 + M])
                    nc.vector.tensor_add(out=c_dst, in0=cf[:],
                                         in1=rsb[:, M:2 * M])
                else:
                    nc.vector.tensor_add(out=c_dst, in0=cf[:],
                                         in1=psum_misc[:, 256:256 + M])
            tc_ = wk.tile([P, M], F32, tag="tc")
            nc.scalar.activation(out=tc_[:], in_=c_dst, func=TANH)
            nc.vector.tensor_mul(out=st[sti][:, 2 * M:3 * M],
                                 in0=gio[:, M:2 * M], in1=tc_[:])       # h
            # fh = h @ W_fh.T
            trh = psT.tile([P, 2 * P], BF16, tag="tr")
            nc.tensor.transpose(out=trh[:P, 0:P],
                                in_=st[sti][:, 2 * M:2 * M + P],
                                identity=ident_b[:])
            nc.tensor.transpose(out=trh[:M - P, P:2 * P],
                                in_=st[sti][:, 2 * M + P:3 * M],
                                identity=ident_b[:])
            hT = wk.tile([P, 2 * P], BF16, tag="hT")
            nc.scalar.copy(out=hT[:, 0:P], in_=trh[:, 0:P])
            nc.scalar.copy(out=hT[:M - P, P:2 * P],
                           in_=trh[:M - P, P:2 * P])
            psum_fh = psE.tile([P, M3], F32, tag="pe")
            nc.tensor.matmul(out=psum_fh[:, 0:M], lhsT=hT[:, 0:P],
                             rhs=w_fh_sb[0][:P, :], start=True, stop=False)
            nc.tensor.matmul(out=psum_fh[:, 0:M], lhsT=hT[:M - P, P:2 * P],
                             rhs=w_fh_sb[1][:M - P, :], start=False, stop=True)
            nc.scalar.copy(out=st[sti][:, 0:M], in_=psum_fh[:, 0:M])
            # output h
            nc.gpsimd.dma_start(out=out_d.ap()[sti * P:(sti + 1) * P, :],
                                in_=st[sti][:, 2 * M:3 * M])

        # ================= main schedule =================
        PF = 8
        tiles = pl.tiles
        n_tiles = len(tiles)
        first_tail = next(i for i, t in enumerate(tiles)
                          if t["kind"] == "tail")
        allreduce_done = [False]

        def maybe_allreduce(i):
            if allreduce_done[0]:
                return
            allreduce_done[0] = True
            nc.gpsimd.collective_compute(
                "AllReduce", mybir.AluOpType.add,
                replica_groups=[list(range(n_cores))],
                ins=[partial_d.ap()], outs=[reduced_d.ap()])
            # load reduced into SBUF tiles
            for rt in range(pl.n_tail_tiles):
                rsb = rsp.tile([P, 2 * M], BF16, tag=f"rsb{rt}")
                rb = pl.tail_rbase[rt]
                nc.sync.dma_start(out=rsb[:],
                                  in_=reduced_d.ap()[rb:rb + P, :])
                for tt in tiles:
                    if tt["kind"] == "tail" and tt["rrt"] == rt:
                        tt["rsb"] = rsb

        for i in range(n_tiles + PF):
            if i < n_tiles:
                emit_loads(tiles[i])
            j = i - PF
            if j >= 0:
                if j == first_tail:
                    maybe_allreduce(j)
                emit_compute(tiles[j])

    nc.compile()
    return nc


# ---------------------------------------------------------------------------
# host-side input packing / output assembly
# ---------------------------------------------------------------------------

def build_inputs(pl, x, W_ioux, b_ioux, W_iouh, b_iouh, W_fx, b_fx, W_fh,
                 b_fh):
    import ml_dtypes
    bf16 = ml_dtypes.bfloat16
    N, IN = x.shape
    M = W_fh.shape[0]
    n_cores = pl.n_cores
    TP = len(pl.xp_cols)
    TE = len(pl.xe_cols)

    xa = np.concatenate([x.astype(np.float32),
                         np.ones((N, 1), np.float32)], axis=1)   # [N, 301]
    xa_b = xa.astype(bf16)

    w_iou = np.zeros((KA, 3 * M), np.float32)
    w_iou[:IN] = W_ioux.T
    w_iou[IN] = b_ioux + b_iouh
    w_f = np.zeros((KA, M), np.float32)
    w_f[:IN] = W_fx.T
    w_f[IN] = b_fx + b_fh
    w_iouh = np.ascontiguousarray(W_iouh.T)
    w_fh = np.ascontiguousarray(W_fh.T)

    def pack_x(cols_list):
        # cols_list: list over tiles of per-core [128] node ids.
        # Output layout: [128 partitions, T*3*128] where partition p holds
        # feature rows {p, 128+p, 256+p} of each tile (chunk-major inside
        # tile) -> one contiguous 2D DMA per tile group.
        T = len(cols_list)
        out = []
        for c in range(n_cores):
            arr = np.zeros((T, KC, P, P), np.float32)   # [t, ci, p, w]
            for ti, cols in enumerate(cols_list):
                ids = cols[c]
                sel = ids >= 0
                if sel.any():
                    block = np.zeros((KPAD, P), np.float32)
                    block[:KA, sel] = xa[ids[sel]].T
                    arr[ti] = block.reshape(KC, P, P)
            out.append(np.ascontiguousarray(
                arr.transpose(2, 0, 1, 3).reshape(P, T * KPAD)).astype(bf16))
        return out

    x_tp_pc = pack_x(pl.xp_cols)
    if TE:
        x_te_pc = pack_x(pl.xe_cols)
    else:
        x_te_pc = [np.zeros((P, KPAD), bf16) for _ in range(n_cores)]

    NT = max(1, pl.n_tables)
    iota_cols = np.tile(np.arange(P, dtype=np.float32)[None, :], (P, 1))
    tb_pc = []
    for c in range(n_cores):
        tbl = np.full((P, NT + P), -1.0, np.float32)
        for col in range(pl.n_tables):
            tbl[:, col] = pl.tables[c][col]
        tbl[:, NT:NT + P] = iota_cols
        tb_pc.append(tbl)

    NPS = max(1, len(pl.scat_cols))
    scat_pc = []
    for c in range(n_cores):
        sc = np.zeros((P, NPS), np.int32)
        sc[:, :] = pl.C_tail + P + np.arange(P)[:, None]  # junk default
        for col, scat in enumerate(pl.scat_cols):
            sc[:, col] = scat[c]
        scat_pc.append(sc)

    in_maps = []
    for c in range(n_cores):
        in_maps.append({
            "x_tp": x_tp_pc[c],
            "x_te": x_te_pc[c],
            "tb": tb_pc[c],
            "scat": scat_pc[c],
            "w_iou": w_iou.astype(bf16),
            "w_f": w_f.astype(bf16),
            "w_iouh": w_iouh.astype(bf16),
            "w_fh": w_fh.astype(bf16),
        })
    return in_maps


def assemble_output(pl, results, M=150):
    N = pl.N
    h_full = np.zeros((N, M), np.float32)
    for c in range(pl.n_cores):
        nodes = np.nonzero(pl.core_of[:N] == c)[0]
        if len(nodes):
            slots = pl.slot_of[c][nodes]
            h_full[nodes] = results[c]["out"][slots].astype(np.float32)
    tnodes = np.nonzero(pl.is_tail)[0]
    if len(tnodes):
        slots = pl.slot_of[0][tnodes]
        h_full[tnodes] = results[0]["out"][slots].astype(np.float32)
    return h_full


_PROFILE_STATE = {"exec_ns": None}


def _install_profile_hook():
    import sys, types
    try:
        import antenv.axon_hooks  # noqa: F401
        return True
    except ImportError:
        pass
    try:
        import antenv
        from trn_agent_boot.trn_boot import _ntff_profile_via_ctypes
    except ImportError:
        return False
    hook = _ntff_profile_via_ctypes("/opt/axon/libaxon_pjrt.so")
    if hook is None:
        return False
    mod = types.ModuleType("antenv.axon_hooks")
    state = {"h": hook}
    mod.set_axon_ntff_profile_hook = lambda h: state.__setitem__("h", h)
    mod.get_axon_ntff_profile_hook = lambda: state["h"]
    sys.modules["antenv.axon_hooks"] = mod
    antenv.axon_hooks = mod
    return True


def kernel(x, W_ioux, b_ioux, W_iouh, b_iouh, W_fx, b_fx, W_fh, b_fh, parents):
    import os
    from concourse import bass_utils

    x = np.asarray(x, np.float32)
    parents_np = np.asarray(parents).astype(np.int64)
    pl = build_plan(parents_np, n_cores=8)
    nc = build_kernel(pl, IN=x.shape[1], M=np.asarray(W_fh).shape[0])
    in_maps = build_inputs(pl, x,
                           np.asarray(W_ioux, np.float32),
                           np.asarray(b_ioux, np.float32),
                           np.asarray(W_iouh, np.float32),
                           np.asarray(b_iouh, np.float32),
                           np.asarray(W_fx, np.float32),
                           np.asarray(b_fx, np.float32),
                           np.asarray(W_fh, np.float32),
                           np.asarray(b_fh, np.float32))
    trace = os.environ.get("TREELSTM_PROFILE", "") == "1"
    if trace:
        trace = _install_profile_hook()
    res = bass_utils.run_bass_kernel_spmd(
        nc, in_maps, core_ids=list(range(8)), trace=trace)
    _PROFILE_STATE["exec_ns"] = res.exec_time_ns
    return assemble_output(pl, res.results).astype(np.float32)
